# revision 17
# baseline (speedup 1.0000x reference)
"""BiLSTM-CRF Trainium2 kernel, v4.

v4 on top of v3 (device exec 9.5ms -> 6.4ms, measured by chaining N
dispatches before one block_until_ready): gates_x = Wih.x + bias is
hoisted out of the recurrence into big n=512 fp32r matmuls per 64-step
chunk, rejoined in-loop via one DVE add per chain - PE (the saturated
engine per CoreSim occupancy) drops from 64 to 32 instructions/step.
Host side: no donation (feats is fully written, so one persistent zeros
set serves every call), and viterbi decodes each core's shard as it
lands, overlapped with the remaining d2h. NOTE: fusing the two chains'
ACT/DVE tails into single ops was TRIED and was 25% SLOWER on HW (lost
inter-chain engine overlap; sim predicted it) - don't redo that.

Architecture vs v2: the bottleneck was the axon PJRT tunnel (~80ms RTT,
~120MB/s), not the device. v3 therefore:
  - keeps the big constants (packed fp16 emb table, f32 weights) DEVICE-
    RESIDENT across calls (fingerprint-checked device_put cache),
  - builds the jax.jit(shard_map) callable ONCE (v2 re-traced per call),
  - gathers embeddings ON DEVICE via gpsimd dma_gather (vocab 50000 >
    int16 so emb rows are packed in pairs: idx = tok>>1 < 25000 selects a
    512B element holding rows [2i, 2i+1]; a parity row then picks
    even/odd via copy_predicated). Gathers are issued in 512-index
    chunks, serialized - >=1024 in-flight descriptors overflow the SWDGE
    ring and wedge the device (bisected empirically; sim passes any size).
  - per-call wire traffic is just 24KB/core of indices+parity (+feats
    fetch); v2 uploaded 23MB per call.

Sharding: 8 cores x 8 sentences, BOTH LSTM directions per core (v2 used
2 dirs x 4 batch quarters). The two direction chains overlap on the
engines like v2's two batch chains did; the backward chain simply
indexes x/h with affine-reversed slices (ts(T-1-iv)), so the one x copy
serves both directions and feats = hf.Wf + hb.Wb + bout completes
per-core. Viterbi stays on host (~30ms numpy).

Device layout is v2's gates-transposed scheme: gates [128 j, batch],
weights stationary as lhsT, 8-col moving rhs; h written straight into
hAll (fwd: slot t+1, bwd: slot t) feeding both the recurrence and the
block emissions matmuls. Gate rows host-permuted to [i, f, o, g].
"""

import numpy as np
from contextlib import ExitStack

import jax
import concourse.bass as bass
import concourse.bacc as bacc
import concourse.tile as tile
from concourse import mybir
from concourse.library_config import mlp

B, T, V, E, H, K = 64, 512, 50000, 128, 256, 9
NCORES = 8
SPC = 8                   # sentences per core
CB = 8                    # batch columns per matmul (= SPC)
NJT = 8                   # j tiles (4H / 128)
KP = 10                   # K padded to even (fp32r moving size must be even)
VP = V // 2               # packed emb rows
GCH = 512                 # gather chunk (SWDGE ring limit: keep <=512 in flight)
F32 = mybir.dt.float32
F32R = mybir.dt.float32r
F16 = mybir.dt.float16
I16 = mybir.dt.int16
U8 = mybir.dt.uint8

OFF_W = 0
OFF_WOUT = 6144
OFF_BS = 6184
OFF_H0 = 6201
OFF_C0 = 6233
AUXW = 6266


def _build_nc(TS=T):
    NT = SPC * TS             # tokens per core
    NBLK = NT // 128
    NGC = max(NT // GCH, 1)
    CH = min(GCH, NT)
    nc = bacc.Bacc()
    embp_d = nc.dram_tensor("embp", [VP, 256], F16, kind="ExternalInput")
    waux_d = nc.dram_tensor("waux", [128, AUXW], F32R, kind="ExternalInput")
    idxp_d = nc.dram_tensor("idxp", [24, NT // 16], I16, kind="ExternalInput")
    feats_d = nc.dram_tensor("feats", [128, NBLK, K], I16,
                             kind="ExternalOutput")

    outer = ExitStack()
    idxt = outer.enter_context(nc.sbuf_tensor("idxt", [128, NT // 16], I16))
    pari = outer.enter_context(nc.sbuf_tensor("pari", [1, NT // 2], I16))
    paru = outer.enter_context(nc.sbuf_tensor("paru", [128, NT], U8))
    xg = outer.enter_context(nc.sbuf_tensor("xg", [128, NGC, 2, CH], F16))
    lio = outer.enter_context(nc.semaphore("lio"))
    g0 = outer.enter_context(nc.semaphore("g0"))
    g1 = outer.enter_context(nc.semaphore("g1"))
    with nc.Block() as block:
        @block.gpsimd
        def _(gp):
            gp.load_library(mlp)
            for k in range(8):
                gp.dma_start(idxt[16 * k:16 * (k + 1), :],
                             idxp_d[0:16, :]).then_inc(lio, 16)
            gp.dma_start(pari[:], idxp_d[16:24, :]).then_inc(lio, 16)
            gp.wait_ge(lio, 16 * 9)
            gp.partition_broadcast(paru[:, :], pari[:].bitcast(U8))
            gs = [g0, g1]
            for k in range(NGC):
                if k >= 1:
                    gp.wait_ge(gs[(k - 1) % 2], 16 * ((k - 1) // 2 + 1))
                gp.dma_gather(
                    xg[:, k, :, :], embp_d[:, :],
                    idxt[:, k * (CH // 16):(k + 1) * (CH // 16)],
                    CH, CH, 256, transpose=True).then_inc(gs[k % 2], 16)
            for s in range(min(2, NGC)):
                gp.wait_ge(gs[s], 16 * ((NGC - 1 - s) // 2 + 1))

    with tile.TileContext(nc) as tc, ExitStack() as ctx:
        const = ctx.enter_context(tc.tile_pool(name="const", bufs=1))
        state = ctx.enter_context(tc.tile_pool(name="state", bufs=1))

        w_sb = const.tile([128, 2, 3, 1024], F32R)
        nc.sync.dma_start(out=w_sb, in_=waux_d[:, OFF_W:OFF_W + 6144])
        wout_sb = const.tile([128, 4, KP], F32R)
        nc.sync.dma_start(out=wout_sb, in_=waux_d[:, OFF_WOUT:OFF_WOUT + 40])
        # bias+bout on one partition, DMA linear order: biasRow[0, p*17+c]
        # = block[p, c]; host packs flat[d*1024+jt*128+p]=bias, [2048:]=bout
        biasRow = const.tile([1, 2176], F32R)
        nc.sync.dma_start(out=biasRow, in_=waux_d[:, OFF_BS:OFF_BS + 17])
        ones_f32 = const.tile([1, 512], F32)
        nc.vector.memset(ones_f32[:], 1.0)
        ones_sb = ones_f32[:].bitcast(F32R)

        xT = state.tile([128, NT], F32R)
        xTr = xT[:]
        with tc.tile_pool(name="cvt", bufs=1) as cvt:
            xSel = cvt.tile([128, NGC, CH], F32)
            xOdd = cvt.tile([128, NGC, CH], F32)
            nc.vector.tensor_copy(out=xSel[:], in_=xg[:, :, 0, :])
            nc.vector.tensor_copy(out=xOdd[:], in_=xg[:, :, 1, :])
            nc.vector.copy_predicated(
                out=xSel[:],
                mask=paru[:, :].rearrange("p (a b) -> p a b", a=NGC),
                data=xOdd[:])
            # fp32r consumers need an fp32r-rounded producer
            nc.vector.tensor_copy(
                out=xT[:].rearrange("p (a b) -> p a b", a=NGC), in_=xSel[:])

        hAll = state.tile([128, 2, 2, (TS + 1) * CB], F32R)
        nc.sync.dma_start(out=hAll[:, 0, :, 0:CB],
                          in_=waux_d[:, OFF_H0:OFF_H0 + 16])
        nc.sync.dma_start(out=hAll[:, 1, :, TS * CB:TS * CB + CB],
                          in_=waux_d[:, OFF_H0 + 16:OFF_H0 + 32])
        c_st = state.tile([128, 2, 2, CB], F32R)
        nc.sync.dma_start(out=c_st, in_=waux_d[:, OFF_C0:OFF_C0 + 32])
        feats_sb = state.tile([128, NBLK, KP], I16)

        # gates_x = Wih.x + bias precomputed per 64-step chunk in big
        # n=512 matmuls (PE was the saturated engine; this halves the
        # per-step PE instruction count - the in-loop PSUM group is just
        # the two Whh matmuls per j-tile, and gates_x joins via one DVE
        # add per chain, DVE having ~6x headroom).
        CST = min(64, TS)                 # steps per gates_x chunk
        CW = CST * CB                     # token columns per chunk
        gx = state.tile([128, 2, NJT, CW], F32)
        gp_ctx = ExitStack()
        gp_pool = gp_ctx.enter_context(
            tc.tile_pool(name="gp", bufs=2, space="PSUM"))
        gx_pool = gp_ctx.enter_context(
            tc.tile_pool(name="gxp", bufs=2, space="PSUM"))
        tmp_pool = ctx.enter_context(tc.tile_pool(name="tmp", bufs=4))

        def step(iv, c0):
            # ch == direction. fwd x col at t=iv, h slots iv -> iv+1;
            # bwd x col at t=TS-1-iv, h slots TS-iv -> TS-1-iv.
            hrd = [bass.ts(iv, CB), bass.ts(TS - iv, CB)]
            hwr = [bass.ts(iv + 1, CB), bass.ts(TS - 1 - iv, CB)]
            gxc = [bass.ts(iv - c0 * CST, CB),
                   bass.ts(c0 * CST + CST - 1 - iv, CB)]
            g = []
            for ch in range(2):
                g_ps = gp_pool.tile([128, NJT, CB], F32, space="PSUM",
                                    tag=f"g{ch}", padded_shape=[128, 8, 64],
                                    name=f"g{ch}")
                for jt in range(NJT):
                    nc.tensor.matmul(
                        out=g_ps[:, jt, :],
                        lhsT=w_sb[:, ch, 1, jt * 128:(jt + 1) * 128],
                        rhs=hAll[:, ch, 0, hrd[ch]],
                        start=(jt == 0), stop=False)
                    nc.tensor.matmul(
                        out=g_ps[:, jt, :],
                        lhsT=w_sb[:, ch, 2, jt * 128:(jt + 1) * 128],
                        rhs=hAll[:, ch, 1, hrd[ch]],
                        start=False, stop=(jt == NJT - 1))
                g.append(g_ps)
            for ch in range(2):
                g_ps = g[ch]
                gsum = tmp_pool.tile([128, NJT, CB], F32, tag=f"gs{ch}")
                nc.vector.tensor_add(gsum[:], g_ps[:, :, :],
                                     gx[:, ch, :, gxc[ch]])
                sg = tmp_pool.tile([128, 6, CB], F32R, tag=f"sg{ch}")
                nc.scalar.activation(
                    out=sg[:], in_=gsum[:, 0:6, :],
                    func=mybir.ActivationFunctionType.Sigmoid)
                tg = tmp_pool.tile([128, 2, CB], F32R, tag=f"tg{ch}")
                nc.scalar.activation(
                    out=tg[:], in_=gsum[:, 6:8, :],
                    func=mybir.ActivationFunctionType.Tanh)
                t1 = tmp_pool.tile([128, 2, CB], F32R, tag=f"t1{ch}")
                t2 = tmp_pool.tile([128, 2, CB], F32R, tag=f"t2{ch}")
                nc.vector.tensor_mul(t1[:], sg[:, 2:4, :], c_st[:, ch, :, :])
                nc.vector.tensor_mul(t2[:], sg[:, 0:2, :], tg[:])
                nc.vector.tensor_add(c_st[:, ch, :, :], t1[:], t2[:])
                th = tmp_pool.tile([128, 2, CB], F32R, tag=f"th{ch}")
                nc.scalar.activation(
                    out=th[:], in_=c_st[:, ch, :, :],
                    func=mybir.ActivationFunctionType.Tanh)
                nc.vector.tensor_mul(
                    hAll[:, ch, :, hwr[ch]], sg[:, 4:6, :], th[:])

        for c0 in range(TS // CST):
            # regen gates_x for steps [c0*CST, (c0+1)*CST). dir1's chunk
            # is stored in forward token order; the loop indexes it with
            # an affine-reversed slice.
            for ch in range(2):
                cw0 = (c0 * CW if ch == 0 else NT - (c0 + 1) * CW)
                for jt in range(NJT):
                    ps = gx_pool.tile([128, CW], F32, space="PSUM",
                                      tag="gx", padded_shape=[128, 512])
                    nc.tensor.matmul(
                        out=ps[:],
                        lhsT=biasRow[:, ch * 1024 + jt * 128:
                                     ch * 1024 + (jt + 1) * 128],
                        rhs=ones_sb[:, 0:CW], start=True, stop=False)
                    nc.tensor.matmul(
                        out=ps[:],
                        lhsT=w_sb[:, ch, 0, jt * 128:(jt + 1) * 128],
                        rhs=xTr[:, cw0:cw0 + CW], start=False, stop=True)
                    nc.vector.tensor_copy(out=gx[:, ch, jt, :], in_=ps[:])
            tc.For_i_unrolled(c0 * CST, (c0 + 1) * CST, 1,
                              lambda iv, c0=c0: step(iv, c0), max_unroll=8)

        gp_ctx.close()
        fp_ctx = ExitStack()
        fp_pool = fp_ctx.enter_context(
            tc.tile_pool(name="fp", bufs=2, space="PSUM"))
        for blk in range(NBLK):
            f_ps = fp_pool.tile([128, KP], F32, space="PSUM", tag="f",
                                padded_shape=[128, 512])
            nc.tensor.matmul(
                out=f_ps[:], lhsT=ones_sb[:, 0:128],
                rhs=biasRow[:, 2048:2048 + KP], start=True, stop=False)
            for ch in range(2):
                off2 = blk * 128 + (CB if ch == 0 else 0)
                for hf in range(2):
                    nc.tensor.matmul(
                        out=f_ps[:], lhsT=hAll[:, ch, hf, off2:off2 + 128],
                        rhs=wout_sb[:, 2 * ch + hf, :],
                        start=False, stop=(ch == 1 and hf == 1))
            # quantize for the wire: int16 at scale 4096 (range +-8,
            # err 1.2e-4 << the fp16-x noise; saturating convert clamps)
            nc.scalar.activation(
                out=feats_sb[:, blk, :], in_=f_ps[:],
                func=mybir.ActivationFunctionType.Copy, scale=4096.0)
        fp_ctx.close()
        nc.sync.dma_start(out=feats_d[:, :, :], in_=feats_sb[:, :, 0:K])
    outer.close()
    nc.compile()
    return nc


# gate-row permutation: torch order (i,f,g,o) -> kernel order (i,f,o,g)
_PERM = np.concatenate([np.arange(0, 512), np.arange(768, 1024),
                        np.arange(512, 768)])

_CONST_NAMES = ("emb", "Wih_f", "Whh_f", "bih_f", "bhh_f",
                "Wih_b", "Whh_b", "bih_b", "bhh_b", "Wout", "bout",
                "h0", "c0")


def _prep_waux(ins):
    """Per-core [128, AUXW] f32 constants pack."""
    Wout = np.asarray(ins["Wout"], np.float32)
    wmats = []
    biases = []
    for d, pre in enumerate(("f", "b")):
        Wih = np.asarray(ins[f"Wih_{pre}"], np.float32)[_PERM]
        Whh = np.asarray(ins[f"Whh_{pre}"], np.float32)[_PERM]
        w = np.empty((128, 3, 1024), np.float32)
        w[:, 0] = Wih.T
        w[:, 1] = Whh.T[0:128]
        w[:, 2] = Whh.T[128:256]
        wmats.append(w)
        biases.append((np.asarray(ins[f"bih_{pre}"], np.float32)
                       + np.asarray(ins[f"bhh_{pre}"], np.float32))[_PERM])
    wout = np.zeros((128, 4, KP), np.float32)
    for d in range(2):
        for hf in range(2):
            wout[:, 2 * d + hf, :K] = Wout[:, d * 256 + hf * 128:
                                           d * 256 + (hf + 1) * 128].T
    bs = np.zeros(128 * 17, np.float32)
    bs[0:1024] = biases[0]
    bs[1024:2048] = biases[1]
    bs[2048:2048 + K] = np.asarray(ins["bout"], np.float32)
    h0 = np.asarray(ins["h0"], np.float32)
    c0 = np.asarray(ins["c0"], np.float32)

    wauxs = []
    for c in range(NCORES):
        sl = slice(c * SPC, (c + 1) * SPC)
        aux = np.zeros((128, AUXW), np.float32)
        aux[:, OFF_W:OFF_W + 6144] = np.stack(wmats, 1).reshape(128, 6144)
        aux[:, OFF_WOUT:OFF_WOUT + 40] = wout.reshape(128, 40)
        aux[:, OFF_BS:OFF_BS + 17] = bs.reshape(128, 17)
        # h0T[p, d, hf, b] = h0[d, c*8+b, hf*128+p]
        aux[:, OFF_H0:OFF_H0 + 32] = h0[:, sl].reshape(
            2, SPC, 2, 128).transpose(3, 0, 2, 1).reshape(128, 32)
        aux[:, OFF_C0:OFF_C0 + 32] = c0[:, sl].reshape(
            2, SPC, 2, 128).transpose(3, 0, 2, 1).reshape(128, 32)
        wauxs.append(aux)
    return np.concatenate(wauxs, 0)              # [8*128, AUXW]


def _prep_idxp(sentence, TS=T):
    """Per-core [24, NT//16] i16 indices+parity, concatenated on axis 0."""
    NT = SPC * TS
    sent = np.asarray(sentence).astype(np.int64)
    out = np.empty((NCORES * 24, NT // 16), np.int16)
    for c in range(NCORES):
        flat = sent[c * SPC:(c + 1) * SPC, :TS].T.reshape(NT)  # i = t*8+b
        idx = (flat >> 1).astype(np.int16)
        par = (flat & 1).astype(np.uint8)
        blk = out[c * 24:(c + 1) * 24]
        blk[0:16] = idx.reshape(NT // 16, 16).T
        blk[16:24] = par.view(np.int16).reshape(8, NT // 16)
    return out


def _feats_from_dev(feats_all, TS=T):
    """[8*128, NBLK, K] -> [B, TS, K]. token p of blk = t*8+b."""
    NBLK = SPC * TS // 128
    feats = np.empty((B, TS, K), np.float32)
    for c in range(NCORES):
        f = feats_all[c * 128:(c + 1) * 128]          # [128, NBLK, K]
        f = f.transpose(1, 0, 2).reshape(TS, SPC, K).transpose(1, 0, 2)
        feats[c * SPC:(c + 1) * SPC] = f
    return feats


def _viterbi_np(feats, mask, start, end, trans):
    Bn, Tn, Kn = feats.shape
    score = start[None] + feats[:, 0]
    hist = np.zeros((Tn - 1, Bn, Kn), np.int64)
    for t in range(1, Tn):
        br = score[:, :, None] + trans[None]
        idx = br.argmax(1)
        nxt = np.take_along_axis(br, idx[:, None, :], 1)[:, 0] + feats[:, t]
        score = np.where(mask[:, t][:, None], nxt, score)
        hist[t - 1] = idx
    score = score + end[None]
    tag = score.argmax(-1)
    tags = np.zeros((Bn, Tn), np.int64)
    tags[:, Tn - 1] = tag
    for t in range(Tn - 2, -1, -1):
        tag = np.take_along_axis(hist[t], tag[:, None], 1)[:, 0]
        tags[:, t] = tag
    return tags.astype(np.int32)


try:
    import numba

    @numba.njit(parallel=True, fastmath=False)
    def _viterbi_nb(feats, mask, start, end, trans):
        Bn, Tn, Kn = feats.shape
        tags = np.empty((Bn, Tn), np.int32)
        for b in numba.prange(Bn):
            hist = np.empty((Tn - 1, Kn), np.int8)
            score = np.empty(Kn, np.float32)
            nxt = np.empty(Kn, np.float32)
            for k in range(Kn):
                score[k] = start[k] + feats[b, 0, k]
            for t in range(1, Tn):
                for kn in range(Kn):
                    bi = 0
                    bv = score[0] + trans[0, kn]
                    for kp in range(1, Kn):
                        v = score[kp] + trans[kp, kn]
                        if v > bv:
                            bv = v
                            bi = kp
                    hist[t - 1, kn] = bi
                    nxt[kn] = bv + feats[b, t, kn]
                if mask[b, t]:
                    for k in range(Kn):
                        score[k] = nxt[k]
            bt = 0
            bv = score[0] + end[0]
            for k in range(1, Kn):
                v = score[k] + end[k]
                if v > bv:
                    bv = v
                    bt = k
            tags[b, Tn - 1] = bt
            for t in range(Tn - 2, -1, -1):
                bt = hist[t, bt]
                tags[b, t] = bt
        return tags

    def _viterbi_host(feats, mask, start, end, trans):
        return _viterbi_nb(np.ascontiguousarray(feats),
                           np.ascontiguousarray(mask), start, end,
                           np.ascontiguousarray(trans))
except Exception:                                     # pragma: no cover
    _viterbi_host = _viterbi_np


class _Runner:
    """Once-per-process compiled PJRT callable + device-resident consts."""

    def __init__(self, TS=T):
        from jax.sharding import Mesh, PartitionSpec, NamedSharding
        from jax.experimental.shard_map import shard_map
        from concourse.bass2jax import (_bass_exec_p, partition_id_tensor,
                                        install_neuronx_cc_hook)
        self.TS = TS
        # Compile the numba viterbi now: if it compiled lazily inside the
        # first kernel() call, the idle gap it creates cools the axon
        # tunnel (~+70ms on the next, measured, call).
        _viterbi_host(np.zeros((2, 3, K), np.float32),
                      np.ones((2, 3), bool), np.zeros(K, np.float32),
                      np.zeros(K, np.float32), np.zeros((K, K), np.float32))
        nc = _build_nc(TS)
        self.nc = nc
        install_neuronx_cc_hook()
        partition_name = (nc.partition_id_tensor.name
                          if nc.partition_id_tensor else None)
        in_names, out_names, out_avals, zero_shapes = [], [], [], []
        for alloc in nc.m.functions[0].allocations:
            if not isinstance(alloc, mybir.MemoryLocationSet):
                continue
            name = alloc.memorylocations[0].name
            if alloc.kind == "ExternalInput":
                if name != partition_name:
                    in_names.append(name)
            elif alloc.kind == "ExternalOutput":
                out_names.append(name)
                shape = tuple(alloc.tensor_shape)
                dtype = mybir.dt.np(alloc.dtype)
                out_avals.append(jax.core.ShapedArray(shape, dtype))
                zero_shapes.append((shape, dtype))
        assert in_names == ["embp", "waux", "idxp"], in_names
        assert out_names == ["feats"], out_names
        n_params = len(in_names)
        in_names_all = in_names + out_names
        dbg_name = None
        if nc.dbg_addr is not None:
            dbg_name = nc.dbg_addr.name
        if partition_name is not None:
            in_names_all.append(partition_name)
        donate = tuple(range(n_params, n_params + len(out_names)))

        def _body(*args):
            operands = list(args)
            if partition_name is not None:
                operands.append(partition_id_tensor())
            outs = _bass_exec_p.bind(
                *operands, out_avals=tuple(out_avals),
                in_names=tuple(in_names_all), out_names=tuple(out_names),
                lowering_input_output_aliases=(), sim_require_finite=True,
                sim_require_nnan=True, nc=nc)
            return tuple(outs)

        assert dbg_name is None, "debug build not supported"
        devices = jax.devices()[:NCORES]
        mesh = Mesh(np.asarray(devices), ("core",))
        self.sh = NamedSharding(mesh, PartitionSpec("core"))
        in_specs = (PartitionSpec("core"),) * (n_params + len(out_names))
        out_specs = (PartitionSpec("core"),) * len(out_names)
        # No donation: the kernel writes every feats element, so the
        # output operand's contents never matter and ONE device-resident
        # zeros set can be reused every call (saves a dispatch per call).
        self.sharded = jax.jit(
            shard_map(_body, mesh=mesh, in_specs=in_specs,
                      out_specs=out_specs, check_rep=False),
            keep_unused=True)
        sh = self.sh
        self.zfn = jax.jit(
            lambda: tuple(jax.numpy.zeros((NCORES * s[0], *s[1:]), d)
                          for s, d in zero_shapes),
            out_shardings=tuple(sh for _ in zero_shapes))
        self.zs = self.zfn()
        # AOT-compile so per-call dispatch skips the jit cache machinery
        gshape = lambda s: (NCORES * s[0], *s[1:])
        avals = [jax.ShapeDtypeStruct((NCORES * VP, 256), np.float16, sharding=sh),
                 jax.ShapeDtypeStruct((NCORES * 128, AUXW), np.float32, sharding=sh),
                 jax.ShapeDtypeStruct((NCORES * 24, SPC * TS // 16), np.int16,
                                      sharding=sh)]
        avals += [jax.ShapeDtypeStruct(gshape(s), d, sharding=sh)
                  for s, d in zero_shapes]
        try:
            self.compiled = self.sharded.lower(*avals).compile()
        except Exception:
            self.compiled = self.sharded
        self.const_key = None
        self.const_dev = None
        self._np_cache = {}

    def to_np(self, name, obj, dtype=None):
        """np.asarray with an identity cache (jax inputs convert once)."""
        hit = self._np_cache.get(name)
        if hit is not None and hit[0] is obj:
            return hit[1]
        a = np.asarray(obj, dtype) if dtype else np.asarray(obj)
        self._np_cache[name] = (obj, a)
        return a

    def ensure_consts(self, ins):
        # id() fast path on the RAW objects: no np.asarray (which would
        # re-fetch device-resident jax inputs every call).
        fast = tuple(id(ins[n]) for n in _CONST_NAMES)
        if self.const_key is not None and self.const_key[0] == fast:
            return
        arrs = [np.asarray(ins[n]) for n in _CONST_NAMES]
        import hashlib
        hsh = hashlib.blake2b()
        for a in arrs:
            hsh.update(np.ascontiguousarray(a).tobytes())
        full = hsh.hexdigest()
        if self.const_key is not None and self.const_key[1] == full:
            self.const_key = (fast, full)
            return
        emb16 = np.asarray(ins["emb"]).astype(np.float16)
        embp = emb16.reshape(VP, 256)
        embp_all = np.broadcast_to(embp, (NCORES, VP, 256)).reshape(
            NCORES * VP, 256)
        waux_all = _prep_waux(ins)
        e_dev = jax.device_put(np.ascontiguousarray(embp_all), self.sh)
        w_dev = jax.device_put(waux_all, self.sh)
        jax.block_until_ready((e_dev, w_dev))
        self.const_dev = (e_dev, w_dev)
        self.const_key = (fast, full)
        # Warm the dispatch/fetch pipeline (allocator + tunnel buffers):
        # the first ~2 rounds after fresh buffers run ~50ms slower.
        NT = SPC * self.TS
        idxp0 = np.zeros((NCORES * 24, NT // 16), np.int16)
        for _ in range(3):
            self._dispatch(idxp0)

    def _dispatch_async(self, idxp):
        i_dev = jax.device_put(idxp, self.sh)
        out = self.compiled(self.const_dev[0], self.const_dev[1], i_dev,
                            *self.zs)
        arr = out[0]
        arr.copy_to_host_async()              # overlap the 8 shard d2h's
        return sorted(arr.addressable_shards,
                      key=lambda s: s.index[0].start or 0)

    def _dispatch(self, idxp):
        shards = self._dispatch_async(idxp)
        return np.concatenate([np.asarray(s.data) for s in shards], 0)

    def run(self, ins):
        self.ensure_consts(ins)
        sent = self.to_np("sentence", ins["sentence"])
        return self._dispatch(_prep_idxp(sent, self.TS))


_RUNNER = None


def _get_runner():
    global _RUNNER
    if _RUNNER is None:
        _RUNNER = _Runner()
    return _RUNNER


def kernel_run(trace=False, **inputs):
    r = _get_runner()
    r.ensure_consts(inputs)
    sent = r.to_np("sentence", inputs["sentence"])
    shards = r._dispatch_async(_prep_idxp(sent, r.TS))
    mask = r.to_np("mask", inputs["mask"])
    start = r.to_np("start", inputs["start"], np.float32)
    end = r.to_np("end", inputs["end"], np.float32)
    trans = r.to_np("trans", inputs["trans"], np.float32)
    TS = r.TS
    tags = np.empty((B, TS), np.int32)
    parts = []
    # decode each core's 8 sentences as its shard lands, overlapping the
    # viterbi with the remaining shards' d2h
    for c, s in enumerate(shards):
        f = np.asarray(s.data)                    # [128, NBLK, K] i16
        fc = np.ascontiguousarray(
            f.transpose(1, 0, 2).reshape(TS, SPC, K).transpose(1, 0, 2)
        ).astype(np.float32)
        fc *= (1.0 / 4096.0)
        tags[c * SPC:(c + 1) * SPC] = _viterbi_host(
            fc, mask[c * SPC:(c + 1) * SPC], start, end, trans)
        parts.append(fc)
    r.zfn()        # fire-and-forget ping: keep the tunnel warm across calls
    return tags, np.concatenate(parts, 0)


def kernel(**inputs):
    tags, _ = kernel_run(trace=False, **inputs)
    return tags


# revision 18
# speedup vs baseline: 1.3906x; 1.3906x over previous
"""BiLSTM-CRF Trainium2 kernel, v4.

v4 on top of v3 (device exec 9.5ms -> 6.4ms, measured by chaining N
dispatches before one block_until_ready): gates_x = Wih.x + bias is
hoisted out of the recurrence into big n=512 fp32r matmuls per 64-step
chunk, rejoined in-loop via one DVE add per chain - PE (the saturated
engine per CoreSim occupancy) drops from 64 to 32 instructions/step.
Host side: no donation (feats is fully written, so one persistent zeros
set serves every call), and viterbi decodes each core's shard as it
lands, overlapped with the remaining d2h. NOTE: fusing the two chains'
ACT/DVE tails into single ops was TRIED and was 25% SLOWER on HW (lost
inter-chain engine overlap; sim predicted it) - don't redo that.

Architecture vs v2: the bottleneck was the axon PJRT tunnel (~80ms RTT,
~120MB/s), not the device. v3 therefore:
  - keeps the big constants (packed fp16 emb table, f32 weights) DEVICE-
    RESIDENT across calls (fingerprint-checked device_put cache),
  - builds the jax.jit(shard_map) callable ONCE (v2 re-traced per call),
  - gathers embeddings ON DEVICE via gpsimd dma_gather (vocab 50000 >
    int16 so emb rows are packed in pairs: idx = tok>>1 < 25000 selects a
    512B element holding rows [2i, 2i+1]; a parity row then picks
    even/odd via copy_predicated). Gathers are issued in 512-index
    chunks, serialized - >=1024 in-flight descriptors overflow the SWDGE
    ring and wedge the device (bisected empirically; sim passes any size).
  - per-call wire traffic is just 24KB/core of indices+parity (+feats
    fetch); v2 uploaded 23MB per call.

Sharding: 8 cores x 8 sentences, BOTH LSTM directions per core (v2 used
2 dirs x 4 batch quarters). The two direction chains overlap on the
engines like v2's two batch chains did; the backward chain simply
indexes x/h with affine-reversed slices (ts(T-1-iv)), so the one x copy
serves both directions and feats = hf.Wf + hb.Wb + bout completes
per-core. Viterbi stays on host (~30ms numpy).

Device layout is v2's gates-transposed scheme: gates [128 j, batch],
weights stationary as lhsT, 8-col moving rhs; h written straight into
hAll (fwd: slot t+1, bwd: slot t) feeding both the recurrence and the
block emissions matmuls. Gate rows host-permuted to [i, f, o, g].
"""

import numpy as np
from contextlib import ExitStack

import jax
import concourse.bass as bass
import concourse.bacc as bacc
import concourse.tile as tile
from concourse import mybir
from concourse.library_config import mlp

B, T, V, E, H, K = 64, 512, 50000, 128, 256, 9
NCORES = 8
SPC = 8                   # sentences per core
CB = 8                    # batch columns per matmul (= SPC)
NJT = 8                   # j tiles (4H / 128)
KP = 10                   # K padded to even (fp32r moving size must be even)
VP = V // 2               # packed emb rows
GCH = 512                 # gather chunk (SWDGE ring limit: keep <=512 in flight)
F32 = mybir.dt.float32
F32R = mybir.dt.float32r
F16 = mybir.dt.float16
I16 = mybir.dt.int16
U8 = mybir.dt.uint8

OFF_W = 0
OFF_WOUT = 6144
OFF_BS = 6184
OFF_H0 = 6201
OFF_C0 = 6233
AUXW = 6266


def _build_nc(TS=T):
    NT = SPC * TS             # tokens per core
    NBLK = NT // 128
    NGC = max(NT // GCH, 1)
    CH = min(GCH, NT)
    nc = bacc.Bacc()
    embp_d = nc.dram_tensor("embp", [VP, 256], F16, kind="ExternalInput")
    waux_d = nc.dram_tensor("waux", [128, AUXW], F32R, kind="ExternalInput")
    idxp_d = nc.dram_tensor("idxp", [24, NT // 16], I16, kind="ExternalInput")
    feats_d = nc.dram_tensor("feats", [128, NBLK, K], F32,
                             kind="ExternalOutput")

    outer = ExitStack()
    idxt = outer.enter_context(nc.sbuf_tensor("idxt", [128, NT // 16], I16))
    pari = outer.enter_context(nc.sbuf_tensor("pari", [1, NT // 2], I16))
    paru = outer.enter_context(nc.sbuf_tensor("paru", [128, NT], U8))
    xg = outer.enter_context(nc.sbuf_tensor("xg", [128, NGC, 2, CH], F16))
    lio = outer.enter_context(nc.semaphore("lio"))
    g0 = outer.enter_context(nc.semaphore("g0"))
    g1 = outer.enter_context(nc.semaphore("g1"))
    with nc.Block() as block:
        @block.gpsimd
        def _(gp):
            gp.load_library(mlp)
            for k in range(8):
                gp.dma_start(idxt[16 * k:16 * (k + 1), :],
                             idxp_d[0:16, :]).then_inc(lio, 16)
            gp.dma_start(pari[:], idxp_d[16:24, :]).then_inc(lio, 16)
            gp.wait_ge(lio, 16 * 9)
            gp.partition_broadcast(paru[:, :], pari[:].bitcast(U8))
            gs = [g0, g1]
            for k in range(NGC):
                if k >= 1:
                    gp.wait_ge(gs[(k - 1) % 2], 16 * ((k - 1) // 2 + 1))
                gp.dma_gather(
                    xg[:, k, :, :], embp_d[:, :],
                    idxt[:, k * (CH // 16):(k + 1) * (CH // 16)],
                    CH, CH, 256, transpose=True).then_inc(gs[k % 2], 16)
            for s in range(min(2, NGC)):
                gp.wait_ge(gs[s], 16 * ((NGC - 1 - s) // 2 + 1))

    with tile.TileContext(nc) as tc, ExitStack() as ctx:
        const = ctx.enter_context(tc.tile_pool(name="const", bufs=1))
        state = ctx.enter_context(tc.tile_pool(name="state", bufs=1))

        w_sb = const.tile([128, 2, 3, 1024], F32R)
        nc.sync.dma_start(out=w_sb, in_=waux_d[:, OFF_W:OFF_W + 6144])
        wout_sb = const.tile([128, 4, KP], F32R)
        nc.sync.dma_start(out=wout_sb, in_=waux_d[:, OFF_WOUT:OFF_WOUT + 40])
        # bias+bout on one partition, DMA linear order: biasRow[0, p*17+c]
        # = block[p, c]; host packs flat[d*1024+jt*128+p]=bias, [2048:]=bout
        biasRow = const.tile([1, 2176], F32R)
        nc.sync.dma_start(out=biasRow, in_=waux_d[:, OFF_BS:OFF_BS + 17])
        ones_f32 = const.tile([1, 512], F32)
        nc.vector.memset(ones_f32[:], 1.0)
        ones_sb = ones_f32[:].bitcast(F32R)

        xT = state.tile([128, NT], F32R)
        xTr = xT[:]
        with tc.tile_pool(name="cvt", bufs=1) as cvt:
            xSel = cvt.tile([128, NGC, CH], F32)
            xOdd = cvt.tile([128, NGC, CH], F32)
            nc.vector.tensor_copy(out=xSel[:], in_=xg[:, :, 0, :])
            nc.vector.tensor_copy(out=xOdd[:], in_=xg[:, :, 1, :])
            nc.vector.copy_predicated(
                out=xSel[:],
                mask=paru[:, :].rearrange("p (a b) -> p a b", a=NGC),
                data=xOdd[:])
            # fp32r consumers need an fp32r-rounded producer
            nc.vector.tensor_copy(
                out=xT[:].rearrange("p (a b) -> p a b", a=NGC), in_=xSel[:])

        hAll = state.tile([128, 2, 2, (TS + 1) * CB], F32R)
        nc.sync.dma_start(out=hAll[:, 0, :, 0:CB],
                          in_=waux_d[:, OFF_H0:OFF_H0 + 16])
        nc.sync.dma_start(out=hAll[:, 1, :, TS * CB:TS * CB + CB],
                          in_=waux_d[:, OFF_H0 + 16:OFF_H0 + 32])
        c_st = state.tile([128, 2, 2, CB], F32R)
        nc.sync.dma_start(out=c_st, in_=waux_d[:, OFF_C0:OFF_C0 + 32])
        feats_sb = state.tile([128, NBLK, KP], F32)

        # gates_x = Wih.x + bias precomputed per 64-step chunk in big
        # n=512 matmuls (PE was the saturated engine; this halves the
        # per-step PE instruction count - the in-loop PSUM group is just
        # the two Whh matmuls per j-tile, and gates_x joins via one DVE
        # add per chain, DVE having ~6x headroom).
        CST = min(64, TS)                 # steps per gates_x chunk
        CW = CST * CB                     # token columns per chunk
        gx = state.tile([128, 2, NJT, CW], F32)
        gp_ctx = ExitStack()
        gp_pool = gp_ctx.enter_context(
            tc.tile_pool(name="gp", bufs=2, space="PSUM"))
        gx_pool = gp_ctx.enter_context(
            tc.tile_pool(name="gxp", bufs=2, space="PSUM"))
        tmp_pool = ctx.enter_context(tc.tile_pool(name="tmp", bufs=4))

        def step(iv, c0):
            # ch == direction. fwd x col at t=iv, h slots iv -> iv+1;
            # bwd x col at t=TS-1-iv, h slots TS-iv -> TS-1-iv.
            hrd = [bass.ts(iv, CB), bass.ts(TS - iv, CB)]
            hwr = [bass.ts(iv + 1, CB), bass.ts(TS - 1 - iv, CB)]
            gxc = [bass.ts(iv - c0 * CST, CB),
                   bass.ts(c0 * CST + CST - 1 - iv, CB)]
            g = []
            for ch in range(2):
                g_ps = gp_pool.tile([128, NJT, CB], F32, space="PSUM",
                                    tag=f"g{ch}", padded_shape=[128, 8, 64],
                                    name=f"g{ch}")
                for jt in range(NJT):
                    nc.tensor.matmul(
                        out=g_ps[:, jt, :],
                        lhsT=w_sb[:, ch, 1, jt * 128:(jt + 1) * 128],
                        rhs=hAll[:, ch, 0, hrd[ch]],
                        start=(jt == 0), stop=False)
                    nc.tensor.matmul(
                        out=g_ps[:, jt, :],
                        lhsT=w_sb[:, ch, 2, jt * 128:(jt + 1) * 128],
                        rhs=hAll[:, ch, 1, hrd[ch]],
                        start=False, stop=(jt == NJT - 1))
                g.append(g_ps)
            for ch in range(2):
                g_ps = g[ch]
                gsum = tmp_pool.tile([128, NJT, CB], F32, tag=f"gs{ch}")
                nc.vector.tensor_add(gsum[:], g_ps[:, :, :],
                                     gx[:, ch, :, gxc[ch]])
                sg = tmp_pool.tile([128, 6, CB], F32R, tag=f"sg{ch}")
                nc.scalar.activation(
                    out=sg[:], in_=gsum[:, 0:6, :],
                    func=mybir.ActivationFunctionType.Sigmoid)
                tg = tmp_pool.tile([128, 2, CB], F32R, tag=f"tg{ch}")
                nc.scalar.activation(
                    out=tg[:], in_=gsum[:, 6:8, :],
                    func=mybir.ActivationFunctionType.Tanh)
                t1 = tmp_pool.tile([128, 2, CB], F32R, tag=f"t1{ch}")
                t2 = tmp_pool.tile([128, 2, CB], F32R, tag=f"t2{ch}")
                nc.vector.tensor_mul(t1[:], sg[:, 2:4, :], c_st[:, ch, :, :])
                nc.vector.tensor_mul(t2[:], sg[:, 0:2, :], tg[:])
                nc.vector.tensor_add(c_st[:, ch, :, :], t1[:], t2[:])
                th = tmp_pool.tile([128, 2, CB], F32R, tag=f"th{ch}")
                nc.scalar.activation(
                    out=th[:], in_=c_st[:, ch, :, :],
                    func=mybir.ActivationFunctionType.Tanh)
                nc.vector.tensor_mul(
                    hAll[:, ch, :, hwr[ch]], sg[:, 4:6, :], th[:])

        for c0 in range(TS // CST):
            # regen gates_x for steps [c0*CST, (c0+1)*CST). dir1's chunk
            # is stored in forward token order; the loop indexes it with
            # an affine-reversed slice.
            for ch in range(2):
                cw0 = (c0 * CW if ch == 0 else NT - (c0 + 1) * CW)
                for jt in range(NJT):
                    ps = gx_pool.tile([128, CW], F32, space="PSUM",
                                      tag="gx", padded_shape=[128, 512])
                    nc.tensor.matmul(
                        out=ps[:],
                        lhsT=biasRow[:, ch * 1024 + jt * 128:
                                     ch * 1024 + (jt + 1) * 128],
                        rhs=ones_sb[:, 0:CW], start=True, stop=False)
                    nc.tensor.matmul(
                        out=ps[:],
                        lhsT=w_sb[:, ch, 0, jt * 128:(jt + 1) * 128],
                        rhs=xTr[:, cw0:cw0 + CW], start=False, stop=True)
                    nc.vector.tensor_copy(out=gx[:, ch, jt, :], in_=ps[:])
            tc.For_i_unrolled(c0 * CST, (c0 + 1) * CST, 1,
                              lambda iv, c0=c0: step(iv, c0), max_unroll=8)

        gp_ctx.close()
        fp_ctx = ExitStack()
        fp_pool = fp_ctx.enter_context(
            tc.tile_pool(name="fp", bufs=2, space="PSUM"))
        for blk in range(NBLK):
            f_ps = fp_pool.tile([128, KP], F32, space="PSUM", tag="f",
                                padded_shape=[128, 512])
            nc.tensor.matmul(
                out=f_ps[:], lhsT=ones_sb[:, 0:128],
                rhs=biasRow[:, 2048:2048 + KP], start=True, stop=False)
            for ch in range(2):
                off2 = blk * 128 + (CB if ch == 0 else 0)
                for hf in range(2):
                    nc.tensor.matmul(
                        out=f_ps[:], lhsT=hAll[:, ch, hf, off2:off2 + 128],
                        rhs=wout_sb[:, 2 * ch + hf, :],
                        start=False, stop=(ch == 1 and hf == 1))
            nc.vector.tensor_copy(out=feats_sb[:, blk, :], in_=f_ps[:])
        fp_ctx.close()
        nc.sync.dma_start(out=feats_d[:, :, :], in_=feats_sb[:, :, 0:K])
    outer.close()
    nc.compile()
    return nc


# gate-row permutation: torch order (i,f,g,o) -> kernel order (i,f,o,g)
_PERM = np.concatenate([np.arange(0, 512), np.arange(768, 1024),
                        np.arange(512, 768)])

_CONST_NAMES = ("emb", "Wih_f", "Whh_f", "bih_f", "bhh_f",
                "Wih_b", "Whh_b", "bih_b", "bhh_b", "Wout", "bout",
                "h0", "c0")


def _prep_waux(ins):
    """Per-core [128, AUXW] f32 constants pack."""
    Wout = np.asarray(ins["Wout"], np.float32)
    wmats = []
    biases = []
    for d, pre in enumerate(("f", "b")):
        Wih = np.asarray(ins[f"Wih_{pre}"], np.float32)[_PERM]
        Whh = np.asarray(ins[f"Whh_{pre}"], np.float32)[_PERM]
        w = np.empty((128, 3, 1024), np.float32)
        w[:, 0] = Wih.T
        w[:, 1] = Whh.T[0:128]
        w[:, 2] = Whh.T[128:256]
        wmats.append(w)
        biases.append((np.asarray(ins[f"bih_{pre}"], np.float32)
                       + np.asarray(ins[f"bhh_{pre}"], np.float32))[_PERM])
    wout = np.zeros((128, 4, KP), np.float32)
    for d in range(2):
        for hf in range(2):
            wout[:, 2 * d + hf, :K] = Wout[:, d * 256 + hf * 128:
                                           d * 256 + (hf + 1) * 128].T
    bs = np.zeros(128 * 17, np.float32)
    bs[0:1024] = biases[0]
    bs[1024:2048] = biases[1]
    bs[2048:2048 + K] = np.asarray(ins["bout"], np.float32)
    h0 = np.asarray(ins["h0"], np.float32)
    c0 = np.asarray(ins["c0"], np.float32)

    wauxs = []
    for c in range(NCORES):
        sl = slice(c * SPC, (c + 1) * SPC)
        aux = np.zeros((128, AUXW), np.float32)
        aux[:, OFF_W:OFF_W + 6144] = np.stack(wmats, 1).reshape(128, 6144)
        aux[:, OFF_WOUT:OFF_WOUT + 40] = wout.reshape(128, 40)
        aux[:, OFF_BS:OFF_BS + 17] = bs.reshape(128, 17)
        # h0T[p, d, hf, b] = h0[d, c*8+b, hf*128+p]
        aux[:, OFF_H0:OFF_H0 + 32] = h0[:, sl].reshape(
            2, SPC, 2, 128).transpose(3, 0, 2, 1).reshape(128, 32)
        aux[:, OFF_C0:OFF_C0 + 32] = c0[:, sl].reshape(
            2, SPC, 2, 128).transpose(3, 0, 2, 1).reshape(128, 32)
        wauxs.append(aux)
    return np.concatenate(wauxs, 0)              # [8*128, AUXW]


def _prep_idxp(sentence, TS=T):
    """Per-core [24, NT//16] i16 indices+parity, concatenated on axis 0."""
    NT = SPC * TS
    sent = np.asarray(sentence).astype(np.int64)
    out = np.empty((NCORES * 24, NT // 16), np.int16)
    for c in range(NCORES):
        flat = sent[c * SPC:(c + 1) * SPC, :TS].T.reshape(NT)  # i = t*8+b
        idx = (flat >> 1).astype(np.int16)
        par = (flat & 1).astype(np.uint8)
        blk = out[c * 24:(c + 1) * 24]
        blk[0:16] = idx.reshape(NT // 16, 16).T
        blk[16:24] = par.view(np.int16).reshape(8, NT // 16)
    return out


def _feats_from_dev(feats_all, TS=T):
    """[8*128, NBLK, K] -> [B, TS, K]. token p of blk = t*8+b."""
    NBLK = SPC * TS // 128
    feats = np.empty((B, TS, K), np.float32)
    for c in range(NCORES):
        f = feats_all[c * 128:(c + 1) * 128]          # [128, NBLK, K]
        f = f.transpose(1, 0, 2).reshape(TS, SPC, K).transpose(1, 0, 2)
        feats[c * SPC:(c + 1) * SPC] = f
    return feats


def _viterbi_np(feats, mask, start, end, trans):
    Bn, Tn, Kn = feats.shape
    score = start[None] + feats[:, 0]
    hist = np.zeros((Tn - 1, Bn, Kn), np.int64)
    for t in range(1, Tn):
        br = score[:, :, None] + trans[None]
        idx = br.argmax(1)
        nxt = np.take_along_axis(br, idx[:, None, :], 1)[:, 0] + feats[:, t]
        score = np.where(mask[:, t][:, None], nxt, score)
        hist[t - 1] = idx
    score = score + end[None]
    tag = score.argmax(-1)
    tags = np.zeros((Bn, Tn), np.int64)
    tags[:, Tn - 1] = tag
    for t in range(Tn - 2, -1, -1):
        tag = np.take_along_axis(hist[t], tag[:, None], 1)[:, 0]
        tags[:, t] = tag
    return tags.astype(np.int32)


try:
    import numba

    @numba.njit(parallel=True, fastmath=False)
    def _viterbi_nb(feats, mask, start, end, trans):
        Bn, Tn, Kn = feats.shape
        tags = np.empty((Bn, Tn), np.int32)
        for b in numba.prange(Bn):
            hist = np.empty((Tn - 1, Kn), np.int8)
            score = np.empty(Kn, np.float32)
            nxt = np.empty(Kn, np.float32)
            for k in range(Kn):
                score[k] = start[k] + feats[b, 0, k]
            for t in range(1, Tn):
                for kn in range(Kn):
                    bi = 0
                    bv = score[0] + trans[0, kn]
                    for kp in range(1, Kn):
                        v = score[kp] + trans[kp, kn]
                        if v > bv:
                            bv = v
                            bi = kp
                    hist[t - 1, kn] = bi
                    nxt[kn] = bv + feats[b, t, kn]
                if mask[b, t]:
                    for k in range(Kn):
                        score[k] = nxt[k]
            bt = 0
            bv = score[0] + end[0]
            for k in range(1, Kn):
                v = score[k] + end[k]
                if v > bv:
                    bv = v
                    bt = k
            tags[b, Tn - 1] = bt
            for t in range(Tn - 2, -1, -1):
                bt = hist[t, bt]
                tags[b, t] = bt
        return tags

    def _viterbi_host(feats, mask, start, end, trans):
        return _viterbi_nb(np.ascontiguousarray(feats),
                           np.ascontiguousarray(mask), start, end,
                           np.ascontiguousarray(trans))
except Exception:                                     # pragma: no cover
    _viterbi_host = _viterbi_np


class _Runner:
    """Once-per-process compiled PJRT callable + device-resident consts."""

    def __init__(self, TS=T):
        from jax.sharding import Mesh, PartitionSpec, NamedSharding
        from jax.experimental.shard_map import shard_map
        from concourse.bass2jax import (_bass_exec_p, partition_id_tensor,
                                        install_neuronx_cc_hook)
        self.TS = TS
        # Compile the numba viterbi now: if it compiled lazily inside the
        # first kernel() call, the idle gap it creates cools the axon
        # tunnel (~+70ms on the next, measured, call).
        _viterbi_host(np.zeros((2, 3, K), np.float32),
                      np.ones((2, 3), bool), np.zeros(K, np.float32),
                      np.zeros(K, np.float32), np.zeros((K, K), np.float32))
        nc = _build_nc(TS)
        self.nc = nc
        install_neuronx_cc_hook()
        partition_name = (nc.partition_id_tensor.name
                          if nc.partition_id_tensor else None)
        in_names, out_names, out_avals, zero_shapes = [], [], [], []
        for alloc in nc.m.functions[0].allocations:
            if not isinstance(alloc, mybir.MemoryLocationSet):
                continue
            name = alloc.memorylocations[0].name
            if alloc.kind == "ExternalInput":
                if name != partition_name:
                    in_names.append(name)
            elif alloc.kind == "ExternalOutput":
                out_names.append(name)
                shape = tuple(alloc.tensor_shape)
                dtype = mybir.dt.np(alloc.dtype)
                out_avals.append(jax.core.ShapedArray(shape, dtype))
                zero_shapes.append((shape, dtype))
        assert in_names == ["embp", "waux", "idxp"], in_names
        assert out_names == ["feats"], out_names
        n_params = len(in_names)
        in_names_all = in_names + out_names
        dbg_name = None
        if nc.dbg_addr is not None:
            dbg_name = nc.dbg_addr.name
        if partition_name is not None:
            in_names_all.append(partition_name)
        donate = tuple(range(n_params, n_params + len(out_names)))

        def _body(*args):
            operands = list(args)
            if partition_name is not None:
                operands.append(partition_id_tensor())
            outs = _bass_exec_p.bind(
                *operands, out_avals=tuple(out_avals),
                in_names=tuple(in_names_all), out_names=tuple(out_names),
                lowering_input_output_aliases=(), sim_require_finite=True,
                sim_require_nnan=True, nc=nc)
            return tuple(outs)

        assert dbg_name is None, "debug build not supported"
        devices = jax.devices()[:NCORES]
        mesh = Mesh(np.asarray(devices), ("core",))
        self.sh = NamedSharding(mesh, PartitionSpec("core"))
        in_specs = (PartitionSpec("core"),) * (n_params + len(out_names))
        out_specs = (PartitionSpec("core"),) * len(out_names)
        # No donation: the kernel writes every feats element, so the
        # output operand's contents never matter and ONE device-resident
        # zeros set can be reused every call (saves a dispatch per call).
        self.sharded = jax.jit(
            shard_map(_body, mesh=mesh, in_specs=in_specs,
                      out_specs=out_specs, check_rep=False),
            keep_unused=True)
        sh = self.sh
        self.zfn = jax.jit(
            lambda: tuple(jax.numpy.zeros((NCORES * s[0], *s[1:]), d)
                          for s, d in zero_shapes),
            out_shardings=tuple(sh for _ in zero_shapes))
        self.zs = self.zfn()
        # AOT-compile so per-call dispatch skips the jit cache machinery
        gshape = lambda s: (NCORES * s[0], *s[1:])
        avals = [jax.ShapeDtypeStruct((NCORES * VP, 256), np.float16, sharding=sh),
                 jax.ShapeDtypeStruct((NCORES * 128, AUXW), np.float32, sharding=sh),
                 jax.ShapeDtypeStruct((NCORES * 24, SPC * TS // 16), np.int16,
                                      sharding=sh)]
        avals += [jax.ShapeDtypeStruct(gshape(s), d, sharding=sh)
                  for s, d in zero_shapes]
        try:
            self.compiled = self.sharded.lower(*avals).compile()
        except Exception:
            self.compiled = self.sharded
        self.const_key = None
        self.const_dev = None
        self._np_cache = {}

    def to_np(self, name, obj, dtype=None):
        """np.asarray with an identity cache (jax inputs convert once)."""
        hit = self._np_cache.get(name)
        if hit is not None and hit[0] is obj:
            return hit[1]
        a = np.asarray(obj, dtype) if dtype else np.asarray(obj)
        self._np_cache[name] = (obj, a)
        return a

    def ensure_consts(self, ins):
        # id() fast path on the RAW objects: no np.asarray (which would
        # re-fetch device-resident jax inputs every call).
        fast = tuple(id(ins[n]) for n in _CONST_NAMES)
        if self.const_key is not None and self.const_key[0] == fast:
            return
        arrs = [np.asarray(ins[n]) for n in _CONST_NAMES]
        import hashlib
        hsh = hashlib.blake2b()
        for a in arrs:
            hsh.update(np.ascontiguousarray(a).tobytes())
        full = hsh.hexdigest()
        if self.const_key is not None and self.const_key[1] == full:
            self.const_key = (fast, full)
            return
        emb16 = np.asarray(ins["emb"]).astype(np.float16)
        embp = emb16.reshape(VP, 256)
        embp_all = np.broadcast_to(embp, (NCORES, VP, 256)).reshape(
            NCORES * VP, 256)
        waux_all = _prep_waux(ins)
        e_dev = jax.device_put(np.ascontiguousarray(embp_all), self.sh)
        w_dev = jax.device_put(waux_all, self.sh)
        jax.block_until_ready((e_dev, w_dev))
        self.const_dev = (e_dev, w_dev)
        self.const_key = (fast, full)
        # Warm the dispatch/fetch pipeline (allocator + tunnel buffers):
        # the first ~2 rounds after fresh buffers run ~50ms slower.
        NT = SPC * self.TS
        idxp0 = np.zeros((NCORES * 24, NT // 16), np.int16)
        for _ in range(3):
            self._dispatch(idxp0)

    def _dispatch_async(self, idxp):
        i_dev = jax.device_put(idxp, self.sh)
        out = self.compiled(self.const_dev[0], self.const_dev[1], i_dev,
                            *self.zs)
        arr = out[0]
        arr.copy_to_host_async()              # overlap the 8 shard d2h's
        return sorted(arr.addressable_shards,
                      key=lambda s: s.index[0].start or 0)

    def _dispatch(self, idxp):
        shards = self._dispatch_async(idxp)
        return np.concatenate([np.asarray(s.data) for s in shards], 0)

    def run(self, ins):
        self.ensure_consts(ins)
        sent = self.to_np("sentence", ins["sentence"])
        return self._dispatch(_prep_idxp(sent, self.TS))


_RUNNER = None


def _get_runner():
    global _RUNNER
    if _RUNNER is None:
        _RUNNER = _Runner()
    return _RUNNER


def kernel_run(trace=False, **inputs):
    r = _get_runner()
    r.ensure_consts(inputs)
    sent = r.to_np("sentence", inputs["sentence"])
    shards = r._dispatch_async(_prep_idxp(sent, r.TS))
    mask = r.to_np("mask", inputs["mask"])
    start = r.to_np("start", inputs["start"], np.float32)
    end = r.to_np("end", inputs["end"], np.float32)
    trans = r.to_np("trans", inputs["trans"], np.float32)
    TS = r.TS
    tags = np.empty((B, TS), np.int32)
    parts = []
    # decode each core's 8 sentences as its shard lands, overlapping the
    # viterbi with the remaining shards' d2h
    for c, s in enumerate(shards):
        f = np.asarray(s.data)                    # [128, NBLK, K]
        fc = np.ascontiguousarray(
            f.transpose(1, 0, 2).reshape(TS, SPC, K).transpose(1, 0, 2))
        tags[c * SPC:(c + 1) * SPC] = _viterbi_host(
            fc, mask[c * SPC:(c + 1) * SPC], start, end, trans)
        parts.append(fc)
    r.zfn()        # fire-and-forget ping: keep the tunnel warm across calls
    return tags, np.concatenate(parts, 0)


def kernel(**inputs):
    tags, _ = kernel_run(trace=False, **inputs)
    return tags
